# revision 22
# baseline (speedup 1.0000x reference)
"""Bahdanau (additive) attention kernel for Trainium2, SPMD over 8 NeuronCores.

Problem (hardcoded):
  key   [32, 2048, 512] f32
  value [32, 2048, 512] f32
  query [32, 1, 512]    f32
  mask  [32, 1, 2048]   i32
  Wk [512, 512], bk [512], Wq [512, 512], v [512]  f32

  k  = key @ Wk.T + bk
  e  = tanh(k + query @ Wq.T) @ v
  e  = where(mask == 0, -inf, e)
  aw = softmax(e); cv = aw @ value
  returns (cv [B,1,512], aw [B,1,1,2048])

Sharding: data-parallel over batch B, 4 batches per core.  Small tensors
(query/Wq/bk -> fused bias columns, v -> columns, mask -> additive mask) are
preprocessed on the host; key/value stream through the chip.

Matmul operands are fp16 (PE does fp32 matmuls in two half-rate passes, so
16-bit operands are ~4.7x faster); all accumulation is fp32 in PSUM and the
softmax runs in fp32.  End-to-end error vs the fp32 reference ~4e-4.

Per-core dataflow:
  key --cast-DMA--> fp16 [128n, 512d] --PE transpose--> keyT [128d, n]
  kprojT[a, n] = sum_d WkT[d, a] * keyT[d, n]     (PE, fp32 accum)
  tmpT = tanh(kprojT + bias[a])                   (ACT, fp16 out)
  e[n] = sum_a v[a] * tmpT[a, n]                  (PE)
  row softmax on partitions {0,32,64,96} (one row per batch, fp32)
  aw row --PE transpose--> columns; cv = sum_n aw[n] * value[n, :]  (PE)
"""

import os
from contextlib import ExitStack

import numpy as np

B, KLEN, DIM = 32, 2048, 512
NCORES = 8
NB = B // NCORES          # batches per core = 4
NCHUNK = KLEN // 512      # 512-wide n-chunks per batch = 4
NSUB = 4                  # 128-subtiles per n-chunk
AT = DIM // 128           # a-tiles = 4
DT = DIM // 128           # d-tiles = 4

_CACHE = {}


def _build():
    import concourse.tile as tile
    from concourse import bacc, mybir

    f32 = mybir.dt.float32
    f16 = mybir.dt.float16
    AF = mybir.ActivationFunctionType
    ALU = mybir.AluOpType
    AX = mybir.AxisListType

    nc = bacc.Bacc(
        "TRN2", target_bir_lowering=False, debug=False, num_devices=NCORES
    )

    key = nc.dram_tensor("key", [NB, KLEN, DIM], f16, kind="ExternalInput").ap()
    value = nc.dram_tensor("value", [NB, KLEN, DIM], f16, kind="ExternalInput").ap()
    wkT = nc.dram_tensor("wkT", [DIM, DIM], f16, kind="ExternalInput").ap()
    bias_cols = nc.dram_tensor("bias_cols", [128, AT * NB], f32, kind="ExternalInput").ap()
    vcols = nc.dram_tensor("vcols", [128, AT], f16, kind="ExternalInput").ap()
    maskadd = nc.dram_tensor("maskadd", [NB, KLEN], f32, kind="ExternalInput").ap()
    ident = nc.dram_tensor("ident", [128, 128], f32, kind="ExternalInput").ap()

    cv_out = nc.dram_tensor("cv_out", [NB, DIM], f32, kind="ExternalOutput").ap()
    aw_out = nc.dram_tensor("aw_out", [NB, KLEN], f32, kind="ExternalOutput").ap()

    with tile.TileContext(nc) as tc, ExitStack() as ctx:
        const = ctx.enter_context(tc.tile_pool(name="const", bufs=1))
        rows = ctx.enter_context(tc.tile_pool(name="rows", bufs=1))
        kt_pool = ctx.enter_context(tc.tile_pool(name="kt", bufs=8))
        tmp_pool = ctx.enter_context(tc.tile_pool(name="tmp", bufs=4))
        vnat_pool = ctx.enter_context(tc.tile_pool(name="vnat", bufs=8))
        pcol_sb_pool = ctx.enter_context(tc.tile_pool(name="pcolsb", bufs=2))
        k_psum = ctx.enter_context(tc.tile_pool(name="k_ps", bufs=4, space="PSUM"))
        e_psum = ctx.enter_context(tc.tile_pool(name="e_ps", bufs=1, space="PSUM"))
        pcol_psum = ctx.enter_context(tc.tile_pool(name="pc_ps", bufs=1, space="PSUM"))
        cv_psum = ctx.enter_context(tc.tile_pool(name="cv_ps", bufs=1, space="PSUM"))

        # ---- constants (SWDGE queues; keep the HWDGE rings free for key/value) ----
        wkT_sb = []
        for dt in range(DT):
            t = const.tile([128, DIM], f16, tag=f"wkT{dt}")
            nc.gpsimd.dma_start(t[:], wkT[dt * 128:(dt + 1) * 128, :])
            wkT_sb.append(t)
        ident_sb = const.tile([128, 128], f32, tag="ident")
        nc.gpsimd.dma_start(ident_sb[:], ident[:])
        bias_sb = const.tile([128, AT * NB], f32, tag="bias")
        nc.gpsimd.dma_start(bias_sb[:], bias_cols[:])
        vcols_sb = const.tile([128, AT], f16, tag="vcols")
        nc.gpsimd.dma_start(vcols_sb[:], vcols[:])
        # per-batch rows live on partitions {0, 32, 64, 96}
        maskadd_sb = rows.tile([128, KLEN], f32, tag="maskadd")
        for b in range(NB):
            nc.gpsimd.dma_start(maskadd_sb[32 * b:32 * b + 1, :], maskadd[b:b + 1, :])

        e_sb = rows.tile([128, KLEN], f32, tag="e_sb")
        p_row = rows.tile([128, KLEN], f32, tag="p_row")
        aw_sb = rows.tile([128, KLEN], f32, tag="aw_sb")
        cv_sb = rows.tile([128, DIM], f32, tag="cv_sb")
        pmax = rows.tile([128, NCHUNK], f32, tag="pmax")
        psums = rows.tile([128, NCHUNK], f32, tag="psums")
        mx = rows.tile([128, 1], f32, tag="mx")
        negmax = rows.tile([128, 1], f32, tag="negmax")
        sump = rows.tile([128, 1], f32, tag="sump")
        inv = rows.tile([128, 1], f32, tag="inv")

        # E: slot (b, ncnk) -> partition 32*b, bank ncnk % 2
        e_ps = e_psum.tile([128, 1024], f32, tag="E")
        cv_ps = cv_psum.tile([128, DIM], f32, tag="CV")

        for b in range(NB):
            # keyT[d, n] tiles via xbar DMA-transpose.  All transposes stay on
            # ONE HWDGE ring (the shared xbar corrupts data when transposes
            # run concurrently from both rings); per-chunk pieces keep the PE
            # fed at fine granularity.
            ktb = []
            for dt in range(DT):
                kt = kt_pool.tile([128, KLEN], f16, tag="kT")
                ktb.append(kt)
            for c in range(NCHUNK):
                for dt in range(DT):
                    nc.sync.dma_start_transpose(
                        ktb[dt][:, c * 512:(c + 1) * 512],
                        key[b, c * 512:(c + 1) * 512, dt * 128:(dt + 1) * 128],
                    )
            # value prefetch for this batch's cv (second HWDGE ring)
            vtiles = []
            for vc in range(NCHUNK):
                vt = vnat_pool.tile([128, NSUB, DIM], f16, tag="vnat")
                nc.scalar.dma_start(
                    vt[:],
                    value[b, vc * 512:(vc + 1) * 512, :].rearrange(
                        "(j p) d -> p j d", p=128
                    ),
                )
                vtiles.append(vt)

            for ncnk in range(NCHUNK):
                # kprojT [a, n] + tanh + e row accumulation
                for at in range(AT):
                    k_ps = k_psum.tile([128, 512], f32, tag="K")
                    for dt in range(DT):
                        nc.tensor.matmul(
                            k_ps[:],
                            wkT_sb[dt][:, at * 128:(at + 1) * 128],
                            ktb[dt][:, ncnk * 512:(ncnk + 1) * 512],
                            start=(dt == 0),
                            stop=(dt == DT - 1),
                        )
                    tmpT = tmp_pool.tile([128, 512], f16, tag="tmpT")
                    nc.scalar.activation(
                        tmpT[:],
                        k_ps[:],
                        AF.Tanh,
                        bias=bias_sb[:, at * NB + b:at * NB + b + 1],
                    )
                    nc.tensor.matmul(
                        e_ps[32 * b:32 * b + 1, (ncnk % 2) * 512:(ncnk % 2 + 1) * 512],
                        vcols_sb[:, at:at + 1],
                        tmpT[:],
                        start=(at == 0),
                        stop=(at == AT - 1),
                        tile_position=(0, 32 * b),
                    )
                # evacuate e slot + apply additive mask, then chunk max
                nc.vector.scalar_tensor_tensor(
                    out=e_sb[32 * b:32 * b + 1, ncnk * 512:(ncnk + 1) * 512],
                    in0=e_ps[32 * b:32 * b + 1, (ncnk % 2) * 512:(ncnk % 2 + 1) * 512],
                    scalar=1.0,
                    in1=maskadd_sb[32 * b:32 * b + 1, ncnk * 512:(ncnk + 1) * 512],
                    op0=ALU.mult,
                    op1=ALU.add,
                )
                nc.vector.reduce_max(
                    pmax[32 * b:32 * b + 1, ncnk:ncnk + 1],
                    e_sb[32 * b:32 * b + 1, ncnk * 512:(ncnk + 1) * 512],
                    axis=AX.X,
                )

            # ---- softmax over this batch's row (partition 32*b) ----
            # exp is chunked so the aw transposes and cv matmuls overlap it;
            # normalization by 1/sum is deferred to the cv/aw epilogues.
            r = slice(32 * b, 32 * b + 1)
            nc.vector.reduce_max(mx[r, 0:1], pmax[r, :], axis=AX.X)
            nc.vector.tensor_scalar_mul(negmax[r, 0:1], mx[r, 0:1], -1.0)
            pc_ps = pcol_psum.tile([128, 16], f32, tag="PC")
            pcols = pcol_sb_pool.tile([128, 16], f16, tag="pcols")
            for c in range(NCHUNK):
                nc.scalar.activation(
                    p_row[r, c * 512:(c + 1) * 512],
                    e_sb[r, c * 512:(c + 1) * 512],
                    AF.Exp,
                    bias=negmax[r, 0:1],
                    accum_out=psums[r, c:c + 1],
                )
                for j in range(4):
                    t = c * 4 + j
                    nc.tensor.transpose(
                        pc_ps[:, t:t + 1],
                        p_row[r, t * 128:(t + 1) * 128],
                        ident_sb[r, 32 * b:32 * b + 1],
                        tile_position=(32 * b, 0),
                    )
                nc.vector.tensor_copy(
                    pcols[:, c * 4:(c + 1) * 4], pc_ps[:, c * 4:(c + 1) * 4]
                )
            nc.vector.reduce_sum(sump[r, 0:1], psums[r, :], axis=AX.X)
            nc.vector.reciprocal(inv[r, 0:1], sump[r, 0:1])
            # normalized attention-weight output (overlaps the cv matmuls)
            nc.scalar.activation(
                aw_sb[r, :], p_row[r, :], AF.Copy, bias=0.0, scale=inv[r, 0:1]
            )
            nc.scalar.dma_start(aw_out[b:b + 1, :], aw_sb[r, :])
            # cv matmuls on unnormalized columns
            for t in range(16):
                nc.tensor.matmul(
                    cv_ps[r, :],
                    pcols[:, t:t + 1],
                    vtiles[t // NSUB][:, t % NSUB, :],
                    start=(t == 0),
                    stop=(t == 15),
                    tile_position=(0, 32 * b),
                )
            nc.vector.tensor_scalar_mul(cv_sb[r, :], cv_ps[r, :], inv[r, 0:1])
            nc.sync.dma_start(cv_out[b:b + 1, :], cv_sb[r, :])

    nc.compile()
    return nc


def _prep_inputs(key, value, query, mask, Wk, bk, Wq, v):
    """Host-side prep: shard big tensors, fold small ones into kernel inputs."""
    key = np.ascontiguousarray(np.asarray(key, dtype=np.float16))
    value = np.ascontiguousarray(np.asarray(value, dtype=np.float16))
    query = np.asarray(query, dtype=np.float32)
    mask = np.asarray(mask)
    Wk = np.asarray(Wk, dtype=np.float32)
    bk = np.asarray(bk, dtype=np.float32)
    Wq = np.asarray(Wq, dtype=np.float32)
    v = np.asarray(v, dtype=np.float32)

    qproj = query[:, 0, :] @ Wq.T + bk          # [B, A]
    wkT = np.ascontiguousarray(Wk.T.astype(np.float16))          # [D, A] fp16
    vcols = np.ascontiguousarray(v.reshape(AT, 128).T.astype(np.float16))
    maskadd = (mask[:, 0, :].astype(np.float32) - 1.0) * 1e30   # [B, KLEN]
    ident = np.eye(128, dtype=np.float32)

    in_maps = []
    for c in range(NCORES):
        sl = slice(c * NB, (c + 1) * NB)
        # bias_cols[p, at*NB + b] = qproj[c*NB+b, at*128+p]
        bc = np.ascontiguousarray(
            qproj[sl].reshape(NB, AT, 128).transpose(2, 1, 0).reshape(128, AT * NB)
        )
        in_maps.append(
            {
                "key": key[sl],
                "value": value[sl],
                "wkT": wkT,
                "bias_cols": bc,
                "vcols": vcols,
                "maskadd": np.ascontiguousarray(maskadd[sl]),
                "ident": ident,
            }
        )
    return in_maps


def kernel(key, value, query, mask, Wk, bk, Wq, v):
    from concourse.bass_utils import run_bass_kernel_spmd

    if "nc" not in _CACHE:
        _CACHE["nc"] = _build()
    nc = _CACHE["nc"]

    in_maps = _prep_inputs(key, value, query, mask, Wk, bk, Wq, v)
    res = run_bass_kernel_spmd(
        nc,
        in_maps,
        core_ids=list(range(NCORES)),
        trace=bool(int(os.environ.get("KERNEL_TRACE", "0"))),
    )
    kernel._last_results = res

    cv = np.concatenate([r["cv_out"] for r in res.results]).reshape(B, 1, DIM)
    aw = np.concatenate([r["aw_out"] for r in res.results]).reshape(B, 1, 1, KLEN)
    return cv, aw


# revision 24
# speedup vs baseline: 1.1526x; 1.1526x over previous
"""Bahdanau (additive) attention kernel for Trainium2, SPMD over 8 NeuronCores.

Problem (hardcoded):
  key   [32, 2048, 512] f32
  value [32, 2048, 512] f32
  query [32, 1, 512]    f32
  mask  [32, 1, 2048]   i32
  Wk [512, 512], bk [512], Wq [512, 512], v [512]  f32

  k  = key @ Wk.T + bk
  e  = tanh(k + query @ Wq.T) @ v
  e  = where(mask == 0, -inf, e)
  aw = softmax(e); cv = aw @ value
  returns (cv [B,1,512], aw [B,1,1,2048])

Sharding: data-parallel over batch B, 4 batches per core.  Small tensors
(query/Wq/bk -> fused bias columns, v -> columns, mask -> additive mask) are
preprocessed on the host; key/value stream through the chip.

Matmul operands are fp16 (PE does fp32 matmuls in two half-rate passes, so
16-bit operands are ~4.7x faster); all accumulation is fp32 in PSUM and the
softmax runs in fp32.  End-to-end error vs the fp32 reference ~4e-4.

Per-core dataflow:
  key --cast-DMA--> fp16 [128n, 512d] --PE transpose--> keyT [128d, n]
  kprojT[a, n] = sum_d WkT[d, a] * keyT[d, n]     (PE, fp32 accum)
  tmpT = tanh(kprojT + bias[a])                   (ACT, fp16 out)
  e[n] = sum_a v[a] * tmpT[a, n]                  (PE)
  row softmax on partitions {0,32,64,96} (one row per batch, fp32)
  aw row --PE transpose--> columns; cv = sum_n aw[n] * value[n, :]  (PE)
"""

import os
from contextlib import ExitStack

import numpy as np

B, KLEN, DIM = 32, 2048, 512
NCORES = 8
NB = B // NCORES          # batches per core = 4
NCHUNK = KLEN // 512      # 512-wide n-chunks per batch = 4
NSUB = 4                  # 128-subtiles per n-chunk
AT = DIM // 128           # a-tiles = 4
DT = DIM // 128           # d-tiles = 4

_CACHE = {}


def _build():
    import concourse.tile as tile
    from concourse import bacc, mybir

    f32 = mybir.dt.float32
    f16 = mybir.dt.float16
    AF = mybir.ActivationFunctionType
    ALU = mybir.AluOpType
    AX = mybir.AxisListType

    nc = bacc.Bacc(
        "TRN2", target_bir_lowering=False, debug=False, num_devices=NCORES
    )

    key = nc.dram_tensor("key", [NB, KLEN, DIM], f16, kind="ExternalInput").ap()
    value = nc.dram_tensor("value", [NB, KLEN, DIM], f16, kind="ExternalInput").ap()
    wkT = nc.dram_tensor("wkT", [DIM, DIM], f16, kind="ExternalInput").ap()
    bias_cols = nc.dram_tensor("bias_cols", [128, AT * NB], f32, kind="ExternalInput").ap()
    vcols = nc.dram_tensor("vcols", [128, AT], f16, kind="ExternalInput").ap()
    maskadd = nc.dram_tensor("maskadd", [NB, KLEN], f32, kind="ExternalInput").ap()
    ident = nc.dram_tensor("ident", [128, 128], f32, kind="ExternalInput").ap()

    cv_out = nc.dram_tensor("cv_out", [NB, DIM], f32, kind="ExternalOutput").ap()
    aw_out = nc.dram_tensor("aw_out", [NB, KLEN], f32, kind="ExternalOutput").ap()

    with tile.TileContext(nc) as tc, ExitStack() as ctx:
        const = ctx.enter_context(tc.tile_pool(name="const", bufs=1))
        rows = ctx.enter_context(tc.tile_pool(name="rows", bufs=1))
        kt_pool = ctx.enter_context(tc.tile_pool(name="kt", bufs=8))
        tmp_pool = ctx.enter_context(tc.tile_pool(name="tmp", bufs=4))
        vnat_pool = ctx.enter_context(tc.tile_pool(name="vnat", bufs=8))
        pcol_sb_pool = ctx.enter_context(tc.tile_pool(name="pcolsb", bufs=2))
        k_psum = ctx.enter_context(tc.tile_pool(name="k_ps", bufs=4, space="PSUM"))
        e_psum = ctx.enter_context(tc.tile_pool(name="e_ps", bufs=1, space="PSUM"))
        pcol_psum = ctx.enter_context(tc.tile_pool(name="pc_ps", bufs=1, space="PSUM"))
        cv_psum = ctx.enter_context(tc.tile_pool(name="cv_ps", bufs=1, space="PSUM"))

        # ---- constants (SWDGE queues; keep the HWDGE rings free for key/value) ----
        wkT_sb = []
        for dt in range(DT):
            t = const.tile([128, DIM], f16, tag=f"wkT{dt}")
            nc.gpsimd.dma_start(t[:], wkT[dt * 128:(dt + 1) * 128, :])
            wkT_sb.append(t)
        ident_sb = const.tile([128, 128], f32, tag="ident")
        nc.gpsimd.dma_start(ident_sb[:], ident[:])
        bias_sb = const.tile([128, AT * NB], f32, tag="bias")
        nc.gpsimd.dma_start(bias_sb[:], bias_cols[:])
        vcols_sb = const.tile([128, AT], f16, tag="vcols")
        nc.gpsimd.dma_start(vcols_sb[:], vcols[:])
        # per-batch rows live on partitions {0, 32, 64, 96}
        maskadd_sb = rows.tile([128, KLEN], f32, tag="maskadd")
        for b in range(NB):
            nc.gpsimd.dma_start(maskadd_sb[32 * b:32 * b + 1, :], maskadd[b:b + 1, :])

        e_sb = rows.tile([128, KLEN], f32, tag="e_sb")
        p_row = rows.tile([128, KLEN], f32, tag="p_row")
        aw_sb = rows.tile([128, KLEN], f32, tag="aw_sb")
        cv_sb = rows.tile([128, DIM], f32, tag="cv_sb")
        pmax = rows.tile([128, NCHUNK], f32, tag="pmax")
        psums = rows.tile([128, NCHUNK], f32, tag="psums")
        mx = rows.tile([128, 1], f32, tag="mx")
        negmax = rows.tile([128, 1], f32, tag="negmax")
        sump = rows.tile([128, 1], f32, tag="sump")
        inv = rows.tile([128, 1], f32, tag="inv")

        # E: slot (b, ncnk) -> partition 32*b, bank ncnk % 2
        e_ps = e_psum.tile([128, 1024], f32, tag="E")
        cv_ps = cv_psum.tile([128, DIM], f32, tag="CV")

        for b in range(NB):
            # keyT[d, n] tiles via xbar DMA-transpose.  All transposes stay on
            # ONE HWDGE ring (the shared xbar corrupts data when transposes
            # run concurrently from both rings); per-chunk pieces keep the PE
            # fed at fine granularity.
            ktb = []
            for dt in range(DT):
                kt = kt_pool.tile([128, KLEN], f16, tag="kT")
                ktb.append(kt)
            if b == 0:
                # first batch in halves so the PE starts sooner
                for h in range(2):
                    for dt in range(DT):
                        nc.sync.dma_start_transpose(
                            ktb[dt][:, h * 1024:(h + 1) * 1024],
                            key[b, h * 1024:(h + 1) * 1024, dt * 128:(dt + 1) * 128],
                        )
            else:
                for dt in range(DT):
                    nc.sync.dma_start_transpose(
                        ktb[dt][:], key[b, :, dt * 128:(dt + 1) * 128]
                    )
            # value prefetch for this batch's cv (second HWDGE ring)
            vtiles = []
            for vc in range(NCHUNK):
                vt = vnat_pool.tile([128, NSUB, DIM], f16, tag="vnat")
                nc.scalar.dma_start(
                    vt[:],
                    value[b, vc * 512:(vc + 1) * 512, :].rearrange(
                        "(j p) d -> p j d", p=128
                    ),
                )
                vtiles.append(vt)

            for ncnk in range(NCHUNK):
                # kprojT [a, n] + tanh + e row accumulation
                for at in range(AT):
                    k_ps = k_psum.tile([128, 512], f32, tag="K")
                    for dt in range(DT):
                        nc.tensor.matmul(
                            k_ps[:],
                            wkT_sb[dt][:, at * 128:(at + 1) * 128],
                            ktb[dt][:, ncnk * 512:(ncnk + 1) * 512],
                            start=(dt == 0),
                            stop=(dt == DT - 1),
                        )
                    tmpT = tmp_pool.tile([128, 512], f16, tag="tmpT")
                    nc.scalar.activation(
                        tmpT[:],
                        k_ps[:],
                        AF.Tanh,
                        bias=bias_sb[:, at * NB + b:at * NB + b + 1],
                    )
                    nc.tensor.matmul(
                        e_ps[32 * b:32 * b + 1, (ncnk % 2) * 512:(ncnk % 2 + 1) * 512],
                        vcols_sb[:, at:at + 1],
                        tmpT[:],
                        start=(at == 0),
                        stop=(at == AT - 1),
                        tile_position=(0, 32 * b),
                    )
                # evacuate e slot + apply additive mask, then chunk max
                nc.vector.scalar_tensor_tensor(
                    out=e_sb[32 * b:32 * b + 1, ncnk * 512:(ncnk + 1) * 512],
                    in0=e_ps[32 * b:32 * b + 1, (ncnk % 2) * 512:(ncnk % 2 + 1) * 512],
                    scalar=1.0,
                    in1=maskadd_sb[32 * b:32 * b + 1, ncnk * 512:(ncnk + 1) * 512],
                    op0=ALU.mult,
                    op1=ALU.add,
                )
                nc.vector.reduce_max(
                    pmax[32 * b:32 * b + 1, ncnk:ncnk + 1],
                    e_sb[32 * b:32 * b + 1, ncnk * 512:(ncnk + 1) * 512],
                    axis=AX.X,
                )

            # ---- softmax over this batch's row (partition 32*b) ----
            # exp is chunked so the aw transposes and cv matmuls overlap it;
            # normalization by 1/sum is deferred to the cv/aw epilogues.
            r = slice(32 * b, 32 * b + 1)
            nc.vector.reduce_max(mx[r, 0:1], pmax[r, :], axis=AX.X)
            nc.vector.tensor_scalar_mul(negmax[r, 0:1], mx[r, 0:1], -1.0)
            pc_ps = pcol_psum.tile([128, 16], f32, tag="PC")
            pcols = pcol_sb_pool.tile([128, 16], f16, tag="pcols")
            for c in range(NCHUNK):
                nc.scalar.activation(
                    p_row[r, c * 512:(c + 1) * 512],
                    e_sb[r, c * 512:(c + 1) * 512],
                    AF.Exp,
                    bias=negmax[r, 0:1],
                    accum_out=psums[r, c:c + 1],
                )
                for j in range(4):
                    t = c * 4 + j
                    nc.tensor.transpose(
                        pc_ps[:, t:t + 1],
                        p_row[r, t * 128:(t + 1) * 128],
                        ident_sb[r, 32 * b:32 * b + 1],
                        tile_position=(32 * b, 0),
                    )
                nc.vector.tensor_copy(
                    pcols[:, c * 4:(c + 1) * 4], pc_ps[:, c * 4:(c + 1) * 4]
                )
            nc.vector.reduce_sum(sump[r, 0:1], psums[r, :], axis=AX.X)
            nc.vector.reciprocal(inv[r, 0:1], sump[r, 0:1])
            # normalized attention-weight output (overlaps the cv matmuls)
            nc.scalar.activation(
                aw_sb[r, :], p_row[r, :], AF.Copy, bias=0.0, scale=inv[r, 0:1]
            )
            nc.scalar.dma_start(aw_out[b:b + 1, :], aw_sb[r, :])
            # cv matmuls on unnormalized columns
            for t in range(16):
                nc.tensor.matmul(
                    cv_ps[r, :],
                    pcols[:, t:t + 1],
                    vtiles[t // NSUB][:, t % NSUB, :],
                    start=(t == 0),
                    stop=(t == 15),
                    tile_position=(0, 32 * b),
                )
            nc.vector.tensor_scalar_mul(cv_sb[r, :], cv_ps[r, :], inv[r, 0:1])
            nc.scalar.dma_start(cv_out[b:b + 1, :], cv_sb[r, :])

    nc.compile()
    return nc


def _prep_inputs(key, value, query, mask, Wk, bk, Wq, v):
    """Host-side prep: shard big tensors, fold small ones into kernel inputs."""
    key = np.ascontiguousarray(np.asarray(key, dtype=np.float16))
    value = np.ascontiguousarray(np.asarray(value, dtype=np.float16))
    query = np.asarray(query, dtype=np.float32)
    mask = np.asarray(mask)
    Wk = np.asarray(Wk, dtype=np.float32)
    bk = np.asarray(bk, dtype=np.float32)
    Wq = np.asarray(Wq, dtype=np.float32)
    v = np.asarray(v, dtype=np.float32)

    qproj = query[:, 0, :] @ Wq.T + bk          # [B, A]
    wkT = np.ascontiguousarray(Wk.T.astype(np.float16))          # [D, A] fp16
    vcols = np.ascontiguousarray(v.reshape(AT, 128).T.astype(np.float16))
    maskadd = (mask[:, 0, :].astype(np.float32) - 1.0) * 1e30   # [B, KLEN]
    ident = np.eye(128, dtype=np.float32)

    in_maps = []
    for c in range(NCORES):
        sl = slice(c * NB, (c + 1) * NB)
        # bias_cols[p, at*NB + b] = qproj[c*NB+b, at*128+p]
        bc = np.ascontiguousarray(
            qproj[sl].reshape(NB, AT, 128).transpose(2, 1, 0).reshape(128, AT * NB)
        )
        in_maps.append(
            {
                "key": key[sl],
                "value": value[sl],
                "wkT": wkT,
                "bias_cols": bc,
                "vcols": vcols,
                "maskadd": np.ascontiguousarray(maskadd[sl]),
                "ident": ident,
            }
        )
    return in_maps


def kernel(key, value, query, mask, Wk, bk, Wq, v):
    from concourse.bass_utils import run_bass_kernel_spmd

    if "nc" not in _CACHE:
        _CACHE["nc"] = _build()
    nc = _CACHE["nc"]

    in_maps = _prep_inputs(key, value, query, mask, Wk, bk, Wq, v)
    res = run_bass_kernel_spmd(
        nc,
        in_maps,
        core_ids=list(range(NCORES)),
        trace=bool(int(os.environ.get("KERNEL_TRACE", "0"))),
    )
    kernel._last_results = res

    cv = np.concatenate([r["cv_out"] for r in res.results]).reshape(B, 1, DIM)
    aw = np.concatenate([r["aw_out"] for r in res.results]).reshape(B, 1, 1, KLEN)
    return cv, aw


# revision 29
# speedup vs baseline: 1.5910x; 1.3804x over previous
"""Bahdanau (additive) attention kernel for Trainium2, SPMD over 8 NeuronCores.

Problem (hardcoded):
  key   [32, 2048, 512] f32
  value [32, 2048, 512] f32
  query [32, 1, 512]    f32
  mask  [32, 1, 2048]   i32
  Wk [512, 512], bk [512], Wq [512, 512], v [512]  f32

  k  = key @ Wk.T + bk
  e  = tanh(k + query @ Wq.T) @ v
  e  = where(mask == 0, -inf, e)
  aw = softmax(e); cv = aw @ value
  returns (cv [B,1,512], aw [B,1,1,2048])

Sharding: data-parallel over batch B, 4 batches per core.  Small tensors
(query/Wq/bk -> fused bias columns, v -> columns, mask -> additive mask) are
preprocessed on the host; key/value stream through the chip.

Matmul operands are fp16 (PE does fp32 matmuls in two half-rate passes, so
16-bit operands are ~4.7x faster); all accumulation is fp32 in PSUM and the
softmax runs in fp32.  End-to-end error vs the fp32 reference ~4e-4.

Per-core dataflow:
  key (fp16, host-cast) --xbar DMA-transpose--> keyT [128d, n] in SBUF
  kprojT[a, n] = sum_d WkT[d, a] * keyT[d, n]     (PE, fp32 accum)
  tmpT = tanh(kprojT + bias[a])                   (ACT, fp16 out)
  e[n] = sum_a v[a] * tmpT[a, n]                  (PE)
  row softmax on partitions {0,32,64,96} (one row per batch, fp32)
  aw row --PE transpose--> columns; cv = sum_n aw[n] * value[n, :]  (PE)
"""

import os
from contextlib import ExitStack

import numpy as np

B, KLEN, DIM = 32, 2048, 512
NCORES = 8
NB = B // NCORES          # batches per core = 4
NCHUNK = KLEN // 512      # 512-wide n-chunks per batch = 4
NSUB = 4                  # 128-subtiles per n-chunk
AT = DIM // 128           # a-tiles = 4
DT = DIM // 128           # d-tiles = 4

_CACHE = {}


def _build():
    import concourse.tile as tile
    from concourse import bacc, mybir

    f32 = mybir.dt.float32
    f16 = mybir.dt.float16
    AF = mybir.ActivationFunctionType
    ALU = mybir.AluOpType
    AX = mybir.AxisListType

    nc = bacc.Bacc(
        "TRN2", target_bir_lowering=False, debug=False, num_devices=NCORES
    )

    key = nc.dram_tensor("key", [NB, KLEN, DIM], f16, kind="ExternalInput").ap()
    value = nc.dram_tensor("value", [NB, KLEN, DIM], f16, kind="ExternalInput").ap()
    wkT = nc.dram_tensor("wkT", [DIM, DIM], f16, kind="ExternalInput").ap()
    bias_cols = nc.dram_tensor("bias_cols", [128, AT * NB], f32, kind="ExternalInput").ap()
    vcols = nc.dram_tensor("vcols", [128, AT], f16, kind="ExternalInput").ap()
    maskadd = nc.dram_tensor("maskadd", [NB, KLEN], f32, kind="ExternalInput").ap()
    ident = nc.dram_tensor("ident", [128, 128], f32, kind="ExternalInput").ap()

    cv_out = nc.dram_tensor("cv_out", [NB, DIM], f32, kind="ExternalOutput").ap()
    aw_out = nc.dram_tensor("aw_out", [NB, KLEN], f32, kind="ExternalOutput").ap()

    with tile.TileContext(nc) as tc, ExitStack() as ctx:
        const = ctx.enter_context(tc.tile_pool(name="const", bufs=1))
        rows = ctx.enter_context(tc.tile_pool(name="rows", bufs=1))
        kt_pool = ctx.enter_context(tc.tile_pool(name="kt", bufs=8))
        tmp_pool = ctx.enter_context(tc.tile_pool(name="tmp", bufs=4))
        vnat_pool = ctx.enter_context(tc.tile_pool(name="vnat", bufs=8))
        pcol_sb_pool = ctx.enter_context(tc.tile_pool(name="pcolsb", bufs=2))
        k_psum = ctx.enter_context(tc.tile_pool(name="k_ps", bufs=4, space="PSUM"))
        e_psum = ctx.enter_context(tc.tile_pool(name="e_ps", bufs=1, space="PSUM"))
        pcol_psum = ctx.enter_context(tc.tile_pool(name="pc_ps", bufs=1, space="PSUM"))
        cv_psum = ctx.enter_context(tc.tile_pool(name="cv_ps", bufs=1, space="PSUM"))

        # ---- constants ----
        wkT_sb = []
        for dt in range(DT):
            t = const.tile([128, DIM], f16, tag=f"wkT{dt}")
            nc.sync.dma_start(t[:], wkT[dt * 128:(dt + 1) * 128, :])
            wkT_sb.append(t)
        ident_sb = const.tile([128, 128], f32, tag="ident")
        nc.sync.dma_start(ident_sb[:], ident[:])
        bias_sb = const.tile([128, AT * NB], f32, tag="bias")
        nc.sync.dma_start(bias_sb[:], bias_cols[:])
        vcols_sb = const.tile([128, AT], f16, tag="vcols")
        nc.sync.dma_start(vcols_sb[:], vcols[:])
        # per-batch rows live on partitions {0, 32, 64, 96}
        maskadd_sb = rows.tile([128, KLEN], f32, tag="maskadd")
        for b in range(NB):
            nc.sync.dma_start(maskadd_sb[32 * b:32 * b + 1, :], maskadd[b:b + 1, :])

        e_sb = rows.tile([128, KLEN], f32, tag="e_sb")
        aw_sb = rows.tile([128, KLEN], f32, tag="aw_sb")
        cv_sb = rows.tile([128, DIM], f32, tag="cv_sb")
        mx = rows.tile([128, 1], f32, tag="mx")
        negmax = rows.tile([128, 1], f32, tag="negmax")
        sump = rows.tile([128, 1], f32, tag="sump")
        inv = rows.tile([128, 1], f32, tag="inv")

        # E: slot (b, ncnk) -> partition 32*b, bank ncnk % 2
        e_ps = e_psum.tile([128, 1024], f32, tag="E")
        cv_ps = cv_psum.tile([128, DIM], f32, tag="CV")

        for b in range(NB):
            # keyT[d, n] tiles via xbar DMA-transpose.  All transposes stay
            # on ONE HWDGE ring: the shared xbar corrupts data when
            # transposes run concurrently from both rings.
            ktb = []
            for dt in range(DT):
                kt = kt_pool.tile([128, KLEN], f16, tag="kT")
                nc.sync.dma_start_transpose(
                    kt[:], key[b, :, dt * 128:(dt + 1) * 128]
                )
                ktb.append(kt)
            # value prefetch for this batch's cv (second HWDGE ring)
            vtiles = []
            for vc in range(NCHUNK):
                vt = vnat_pool.tile([128, NSUB, DIM], f16, tag="vnat")
                nc.scalar.dma_start(
                    vt[:],
                    value[b, vc * 512:(vc + 1) * 512, :].rearrange(
                        "(j p) d -> p j d", p=128
                    ),
                )
                vtiles.append(vt)

            for ncnk in range(NCHUNK):
                # kprojT [a, n] + tanh + e row accumulation
                for at in range(AT):
                    k_ps = k_psum.tile([128, 512], f32, tag="K")
                    for dt in range(DT):
                        nc.tensor.matmul(
                            k_ps[:],
                            wkT_sb[dt][:, at * 128:(at + 1) * 128],
                            ktb[dt][:, ncnk * 512:(ncnk + 1) * 512],
                            start=(dt == 0),
                            stop=(dt == DT - 1),
                        )
                    tmpT = tmp_pool.tile([128, 512], f16, tag="tmpT")
                    nc.scalar.activation(
                        tmpT[:],
                        k_ps[:],
                        AF.Tanh,
                        bias=bias_sb[:, at * NB + b:at * NB + b + 1],
                    )
                    nc.tensor.matmul(
                        e_ps[32 * b:32 * b + 1, (ncnk % 2) * 512:(ncnk % 2 + 1) * 512],
                        vcols_sb[:, at:at + 1],
                        tmpT[:],
                        start=(at == 0),
                        stop=(at == AT - 1),
                        tile_position=(0, 32 * b),
                    )
                # evacuate e slot + apply additive mask
                nc.vector.scalar_tensor_tensor(
                    out=e_sb[32 * b:32 * b + 1, ncnk * 512:(ncnk + 1) * 512],
                    in0=e_ps[32 * b:32 * b + 1, (ncnk % 2) * 512:(ncnk % 2 + 1) * 512],
                    scalar=1.0,
                    in1=maskadd_sb[32 * b:32 * b + 1, ncnk * 512:(ncnk + 1) * 512],
                    op0=ALU.mult,
                    op1=ALU.add,
                )

            # ---- softmax over this batch's row (partition 32*b) ----
            r = slice(32 * b, 32 * b + 1)
            nc.vector.reduce_max(mx[r, 0:1], e_sb[r, :], axis=AX.X)
            nc.vector.tensor_scalar_mul(negmax[r, 0:1], mx[r, 0:1], -1.0)
            nc.scalar.activation(
                aw_sb[r, :],
                e_sb[r, :],
                AF.Exp,
                bias=negmax[r, 0:1],
                accum_out=sump[r, 0:1],
            )
            nc.vector.reciprocal(inv[r, 0:1], sump[r, 0:1])
            nc.vector.tensor_scalar_mul(aw_sb[r, :], aw_sb[r, :], inv[r, 0:1])
            nc.sync.dma_start(aw_out[b:b + 1, :], aw_sb[r, :])

            # ---- aw row -> columns (fp16), cv matmuls ----
            pc_ps = pcol_psum.tile([128, 16], f32, tag="PC")
            for t in range(16):
                nc.tensor.transpose(
                    pc_ps[:, t:t + 1],
                    aw_sb[r, t * 128:(t + 1) * 128],
                    ident_sb[r, 32 * b:32 * b + 1],
                    tile_position=(32 * b, 0),
                )
            pcols = pcol_sb_pool.tile([128, 16], f16, tag="pcols")
            nc.any.tensor_copy(pcols[:], pc_ps[:])
            for t in range(16):
                nc.tensor.matmul(
                    cv_ps[r, :],
                    pcols[:, t:t + 1],
                    vtiles[t // NSUB][:, t % NSUB, :],
                    start=(t == 0),
                    stop=(t == 15),
                    tile_position=(0, 32 * b),
                )
            nc.any.tensor_copy(cv_sb[r, :], cv_ps[r, :])
            nc.sync.dma_start(cv_out[b:b + 1, :], cv_sb[r, :])

    nc.compile()
    return nc


def _prep_inputs(key, value, query, mask, Wk, bk, Wq, v):
    """Host-side prep: shard big tensors, fold small ones into kernel inputs."""
    key = np.ascontiguousarray(np.asarray(key, dtype=np.float16))
    value = np.ascontiguousarray(np.asarray(value, dtype=np.float16))
    query = np.asarray(query, dtype=np.float32)
    mask = np.asarray(mask)
    Wk = np.asarray(Wk, dtype=np.float32)
    bk = np.asarray(bk, dtype=np.float32)
    Wq = np.asarray(Wq, dtype=np.float32)
    v = np.asarray(v, dtype=np.float32)

    qproj = query[:, 0, :] @ Wq.T + bk          # [B, A]
    wkT = np.ascontiguousarray(Wk.T.astype(np.float16))          # [D, A] fp16
    vcols = np.ascontiguousarray(v.reshape(AT, 128).T.astype(np.float16))
    maskadd = (mask[:, 0, :].astype(np.float32) - 1.0) * 1e30   # [B, KLEN]
    ident = np.eye(128, dtype=np.float32)

    in_maps = []
    for c in range(NCORES):
        sl = slice(c * NB, (c + 1) * NB)
        # bias_cols[p, at*NB + b] = qproj[c*NB+b, at*128+p]
        bc = np.ascontiguousarray(
            qproj[sl].reshape(NB, AT, 128).transpose(2, 1, 0).reshape(128, AT * NB)
        )
        in_maps.append(
            {
                "key": key[sl],
                "value": value[sl],
                "wkT": wkT,
                "bias_cols": bc,
                "vcols": vcols,
                "maskadd": np.ascontiguousarray(maskadd[sl]),
                "ident": ident,
            }
        )
    return in_maps


def kernel(key, value, query, mask, Wk, bk, Wq, v):
    from concourse.bass_utils import run_bass_kernel_spmd

    if "nc" not in _CACHE:
        _CACHE["nc"] = _build()
    nc = _CACHE["nc"]

    in_maps = _prep_inputs(key, value, query, mask, Wk, bk, Wq, v)
    res = run_bass_kernel_spmd(
        nc,
        in_maps,
        core_ids=list(range(NCORES)),
        trace=bool(int(os.environ.get("KERNEL_TRACE", "0"))),
    )
    kernel._last_results = res

    cv = np.concatenate([r["cv_out"] for r in res.results]).reshape(B, 1, DIM)
    aw = np.concatenate([r["aw_out"] for r in res.results]).reshape(B, 1, 1, KLEN)
    return cv, aw
